# revision 1
# baseline (speedup 1.0000x reference)
"""Multi-head causal attention (B=4, L=2048, E=1024, H=16) on 8 trn2 NeuronCores.

Sharding: (batch, head-group) grid — core c handles batch b=c//2 and heads
g=c%2 (8 heads each).  Each core computes its heads' QKV projection, causal
attention, and a partial output projection; the host sums the two partials
per batch.

v4: bf16 matmuls, head-paired scores, lq-window-512 attention:
  - all matmul operands bf16 (PSUM stays fp32): full-rate at any N, FWL
    weight loads, half DMA.  All DRAM inputs are pre-shuffled on the host to
    the exact SBUF layout so every load is a contiguous full-bandwidth copy.
  - heads 2m / 2m+1 live at partitions 0:64 / 64:128 of q/k slice m; their
    K=64 score matmuls execute CONCURRENTLY in disjoint row halves of the PE
    array (tile_position auto-derives from base_partition).
  - per (window w, lk-tile t): both heads' scores land in ONE [128, 512|512]
    PSUM tile, so a single ACTIVATE exps both heads — the scalar engine is
    the critical resource (~(N+352)/1.2 ns per instr) and this minimizes its
    instruction count.
  - PSUM: sc 2x[128,1024](4 banks) + pv 2x[65,512](2) + heater 2x[128,512](2)
    = 8 banks.  Projection/outproj units run in their own heater pool and
    never stall the exp pipeline; they fill PE slack and keep HAM at 8/8.
  - v l-major [L, 65] per head with a ones column: PV also produces the
    softmax denominator (pv row 64).  exp needs no max subtraction (|s|<~8).
  - softmax normalization is latency-decoupled: at window end only the PSUM
    copy + DMAs are emitted (the sums respread to 128 lanes / broadcast via
    DRAM); the reciprocal and the final multiply are closures popped into
    LATER windows' t-loops, when their DMA inputs are already resident —
    otherwise their semaphore waits convoy the Vector FIFO, which backs up
    the pv-slot WAR, the PE FIFO, and ultimately the exp stream.
"""

import numpy as np

L = 2048
E = 1024
NH = 8        # heads per core
D = 64
JQ = 512      # feature rows per core (NH*D)
ET = E // 128  # 8 e-tiles
LT = L // 128  # 16 l-tiles

_CACHE = {}


def build_nc():
    import concourse.mybir as mybir
    import concourse.tile as tile
    from concourse import bacc
    from contextlib import ExitStack

    f32 = mybir.dt.float32
    bf16 = mybir.dt.bfloat16
    Exp = mybir.ActivationFunctionType.Exp

    nc = bacc.Bacc("TRN2", target_bir_lowering=False, debug=False)

    # all inputs host-pre-shuffled to SBUF layout (partition dim first)
    xT_d = [nc.declare_dram_parameter(f"xT{c}", [128, ET, 512], bf16, isOutput=False)
            for c in range(4)]
    wqkT_d = nc.declare_dram_parameter("wqkT", [128, ET, 2 * JQ], bf16, isOutput=False)
    wvT_d = nc.declare_dram_parameter("wvT", [128, ET, JQ], bf16, isOutput=False)
    woT_d = nc.declare_dram_parameter("woT", [128, 4, E], bf16, isOutput=False)
    diag_d = nc.declare_dram_parameter("diag", [128, 128], bf16, isOutput=False)
    y_d = nc.declare_dram_parameter("y", [L, E], f32, isOutput=True)

    with ExitStack() as ctx:
        tc = ctx.enter_context(tile.TileContext(nc))

        consts = ctx.enter_context(tc.tile_pool(name="consts", bufs=1))
        diag_sb = consts.tile([128, 128], bf16)
        nc.sync.dma_start(out=diag_sb, in_=diag_d.ap())

        wv_p = ctx.enter_context(tc.tile_pool(name="wv", bufs=1))
        wvT_sb = wv_p.tile([128, ET, JQ], bf16)         # 8KB/part
        nc.sync.dma_start(out=wvT_sb, in_=wvT_d.ap())
        xT_p = ctx.enter_context(tc.tile_pool(name="xT", bufs=1))
        xcs = [xT_p.tile([128, ET, 512], bf16, tag=f"xc{c}", name=f"xc{c}")
               for c in range(4)]
        for c in (0, 1):
            nc.sync.dma_start(out=xcs[c], in_=xT_d[c].ap())
        # q/k weights + tail x chunks ride the Activation HWDGE ring: it is
        # idle until the first exp, so these overlap the sync-ring loads
        wqk_p = ctx.enter_context(tc.tile_pool(name="wqk", bufs=1))
        wqkT_sb = wqk_p.tile([128, ET, 2 * JQ], bf16)   # 16KB/part
        nc.scalar.dma_start(out=wqkT_sb, in_=wqkT_d.ap())
        for c in (2, 3):
            nc.scalar.dma_start(out=xcs[c], in_=xT_d[c].ap())

        vaug_p = ctx.enter_context(tc.tile_pool(name="vaug", bufs=1))
        v_aug = vaug_p.tile([128, LT, NH, 65], bf16)    # 16.6KB/part
        nc.vector.memset(v_aug[:, :, :, 64:65], 1.0)

        qk_p = ctx.enter_context(tc.tile_pool(name="qk", bufs=1))
        qT_sb = qk_p.tile([128, 4, L], bf16)            # 16KB/part
        kT_sb = qk_p.tile([128, 4, L], bf16)            # 16KB/part
        ao_p = ctx.enter_context(tc.tile_pool(name="ao", bufs=1))
        aoT_sb = ao_p.tile([128, 4, L], bf16)           # 16KB/part

        sc_pp = ctx.enter_context(tc.tile_pool(name="scpp", bufs=2, space="PSUM"))
        pv_pp = ctx.enter_context(tc.tile_pool(name="pvpp", bufs=2, space="PSUM"))
        hp_pp = ctx.enter_context(tc.tile_pool(name="hppp", bufs=2, space="PSUM"))
        pt_p = ctx.enter_context(tc.tile_pool(name="pt", bufs=12))
        rc_p = ctx.enter_context(tc.tile_pool(name="rc", bufs=2))
        rcd_p = ctx.enter_context(tc.tile_pool(name="rcd", bufs=4, space="DRAM"))
        aou_p = ctx.enter_context(tc.tile_pool(name="aou", bufs=6))
        y_p = ctx.enter_context(tc.tile_pool(name="y", bufs=4))

        dq = []  # deferred normalize closures, popped one per attention t

        # ---- projection / outproj units (heater pool; 1 PSUM bank each) ----
        def v_unit(c, i):
            ps = hp_pp.tile([128, 512], f32, tag="hp", name="vps")
            for et in range(ET):
                nc.tensor.matmul(
                    ps,
                    lhsT=xcs[c][:, et, i * 128:(i + 1) * 128],
                    rhs=wvT_sb[:, et, :],
                    start=(et == 0), stop=(et == ET - 1),
                )
            nc.vector.tensor_copy(
                out=v_aug[:, c * 4 + i, :, 0:64],
                in_=ps.rearrange("p (h d) -> p h d", h=NH),
            )

        def qk_unit(jt, c):
            # jt 0..3 = q j-tiles, 4..7 = k j-tiles
            ps = hp_pp.tile([128, 512], f32, tag="hp", name="qkps")
            dst = qT_sb if jt < 4 else kT_sb
            for et in range(ET):
                nc.tensor.matmul(
                    ps,
                    lhsT=wqkT_sb[:, et, jt * 128:(jt + 1) * 128],
                    rhs=xcs[c][:, et, :],
                    start=(et == 0), stop=(et == ET - 1),
                )
            nc.vector.tensor_copy(out=dst[:, jt % 4, c * 512:(c + 1) * 512], in_=ps)

        def op_unit(lt, ec, ring=None):
            ps = hp_pp.tile([128, 512], f32, tag="hp", name="opps")
            for jt in range(4):
                nc.tensor.matmul(
                    ps,
                    lhsT=aoT_sb[:, jt, lt * 128:(lt + 1) * 128],
                    rhs=woT_sb[:, jt, ec * 512:(ec + 1) * 512],
                    start=(jt == 0), stop=(jt == 3),
                )
            yt = y_p.tile([128, 512], f32, tag="y")
            nc.vector.tensor_copy(out=yt, in_=ps)
            (ring or nc.gpsimd).dma_start(
                out=y_d.ap()[lt * 128:(lt + 1) * 128, ec * 512:(ec + 1) * 512],
                in_=yt,
            )

        # ---- attention ------------------------------------------------
        def pair_unit(m, phase, hw0=(), hw1=(), inline_norm=False):
            """Heads (2m, 2m+1); phase 0 = lq windows 0,1; phase 1 = windows 2,3.

            Head A (partitions 0:64) scores land in sc[:, 0:512], head B
            (64:128) in sc[:, 512:1024]; one ACTIVATE exps both.  heaters
            (hw0/hw1 per window) are drained one per t into their own PSUM
            pool, filling PE slack under the exp stream.
            """
            for w, heaters in ((2 * phase, hw0), (2 * phase + 1, hw1)):
                lq0 = w * 512
                nt = 4 * w + 4
                pvA = pv_pp.tile([65, 512], f32, tag="pv", name="pvA")
                pvB = pv_pp.tile([65, 512], f32, tag="pv", name="pvB")
                hq = list(heaters)
                # PV trails scores by TWO tiles so the first PV of this
                # window issues after the previous window's pv readers have
                # released the slots (else it blocks the PE FIFO)
                pend = []

                def emit_pv(p, stop):
                    pe, poff, tt = p
                    for pv, base, h in ((pvA, 0, 2 * m), (pvB, 512, 2 * m + 1)):
                        nc.tensor.matmul(
                            pv[:, poff:512],
                            lhsT=v_aug[:, tt, h, :],
                            rhs=pe[:, base + poff:base + 512],
                            start=(tt == 0), stop=stop,
                            skip_group_check=True,
                        )

                for t in range(nt):
                    off = max(0, t * 128 - lq0)
                    sc = sc_pp.tile([128, 1024], f32, tag="sc", name="sc")
                    for po, base in ((0, 0), (64, 512)):
                        nc.tensor.matmul(
                            sc[:, base + off:base + 512],
                            lhsT=kT_sb[po:po + 64, m, t * 128:(t + 1) * 128],
                            rhs=qT_sb[po:po + 64, m, lq0 + off:lq0 + 512],
                            start=True, stop=True,
                        )
                    pe = pt_p.tile([128, 1024], bf16, tag="pe", name="pe")
                    nc.scalar.activation(out=pe[:, off:1024], in_=sc[:, off:1024],
                                         func=Exp, scale=0.125)
                    if t >= 4 * w:  # diagonal block: zero lk > lq
                        for base in (0, 512):
                            nc.vector.tensor_mul(
                                out=pe[:, base + off:base + off + 128],
                                in0=pe[:, base + off:base + off + 128],
                                in1=diag_sb,
                            )
                    if dq:
                        dq.pop(0)()
                    if hq:
                        hq.pop(0)()
                    if len(pend) == 2:
                        emit_pv(pend.pop(0), stop=False)
                    pend.append((pe, off, t))
                while pend:
                    emit_pv(pend.pop(0), stop=(len(pend) == 0))
                # normalize: copy PSUM out + all DMAs now (sync ring, fully
                # dependency-ordered); reciprocal and final multiply are
                # deferred into later windows' t-loops so their waits never
                # convoy the Vector FIFO.
                for po, pv, nm in ((0, pvA, "A"), (64, pvB, "B")):
                    aoU = aou_p.tile([65, 512], f32, tag="aou", name="aoU" + nm)
                    nc.vector.tensor_copy(out=aoU, in_=pv)
                    rcd = rcd_p.tile([1, 512], f32, tag="rcd", name="rcd" + nm)
                    nc.sync.dma_start(out=rcd, in_=aoU[64:65, :])
                    rc4 = rc_p.tile([128, 4], f32, tag="rc4", name="rc4" + nm, bufs=4)
                    nc.sync.dma_start(out=rc4, in_=rcd.rearrange("o (p c) -> (o p) c", p=128))
                    rcd2 = rcd_p.tile([1, 512], f32, tag="rcd2", name="rcd2" + nm)
                    rcb = rc_p.tile([64, 512], f32, tag="rcb", name="rcb" + nm, bufs=6)

                    if inline_norm:
                        # last pair: nothing left to convoy — run the whole
                        # chain now so the tail ops' aoT inputs don't lag on
                        # the gpsimd FIFO
                        nc.vector.reciprocal(out=rc4, in_=rc4)
                        nc.sync.dma_start(
                            out=rcd2.rearrange("o (p c) -> (o p) c", p=128), in_=rc4)
                        nc.sync.dma_start(out=rcb, in_=rcd2.to_broadcast((64, 512)))
                        nc.vector.tensor_mul(
                            out=aoT_sb[po:po + 64, m, lq0:lq0 + 512],
                            in0=aoU[0:64, :], in1=rcb,
                        )
                        continue

                    def fin1(rc4=rc4, rcd2=rcd2, rcb=rcb):
                        # back half of the chain rides the gpsimd DGE ring:
                        # on the sync ring its recip-gated descriptors would
                        # head-of-line-block the next windows' rcd/rc4
                        # transfers, and a dma_start costs its issuing engine
                        # ~0.6us — the scalar engine can't spare that
                        nc.vector.reciprocal(out=rc4, in_=rc4)
                        nc.gpsimd.dma_start(
                            out=rcd2.rearrange("o (p c) -> (o p) c", p=128), in_=rc4)
                        nc.gpsimd.dma_start(out=rcb, in_=rcd2.to_broadcast((64, 512)))

                    def fin2(aoU=aoU, rcb=rcb, po=po, m=m, lq0=lq0):
                        # on gpsimd: it has windows of slack (deferred), and
                        # on the Vector FIFO it would queue ahead of the
                        # latency-critical heater casts and aoU copies
                        nc.gpsimd.tensor_mul(
                            out=aoT_sb[po:po + 64, m, lq0:lq0 + 512],
                            in0=aoU[0:64, :], in1=rcb,
                        )

                    dq.append(fin1)
                    dq.append(fin2)

        # ---- schedule -------------------------------------------------
        # warm the HAM clock gate while the x/w DMAs land; the memsets then
        # zero the sc slots so diagonal-strip exps never see raw PSUM
        warmA = sc_pp.tile([128, 1024], f32, tag="sc", name="warmA")
        for _ in range(16):
            nc.tensor.matmul(
                warmA[:, 0:128], lhsT=diag_sb, rhs=diag_sb,
                start=True, stop=True, skip_group_check=True,
            )
        nc.vector.memset(warmA, 0.0)
        warmB = sc_pp.tile([128, 1024], f32, tag="sc", name="warmB")
        nc.vector.memset(warmB, 0.0)

        # P0: only pair-0-window-0's prerequisites run serially; everything
        # else overlaps attention as heaters
        for i in range(4):
            v_unit(0, i)
        qk_unit(0, 0)
        qk_unit(4, 0)

        QK = lambda jt, c: (lambda: qk_unit(jt, c))
        VU = lambda c, i: (lambda: v_unit(c, i))
        OP = lambda lt, ec: (lambda: op_unit(lt, ec))

        # A-phase: windows 0,1; heaters finish the q/k projection
        pair_unit(0, 0, [QK(0, 1), QK(4, 1), VU(1, 0), VU(1, 1)],
                  [QK(5, 0), QK(1, 0), VU(1, 2), VU(1, 3), QK(5, 1), QK(1, 1)])
        pair_unit(1, 0, [QK(6, 0), QK(2, 0)], [QK(6, 1), QK(2, 1)])
        pair_unit(2, 0, [QK(7, 0), QK(3, 0)], [QK(7, 1), QK(3, 1)])
        pair_unit(3, 0, [QK(0, 2), QK(0, 3), QK(4, 2)],
                  [QK(4, 3), QK(1, 2), QK(1, 3), QK(5, 2), QK(5, 3)])

        # B-phase: windows 2,3; heaters: v for lk>=1024, remaining q/k,
        # then the output projection as soon as its aoT rows are final
        woT_sb = wqk_p.tile([128, 4, E], bf16, tag="wqkT_sb", name="woT_sb")

        def load_wo():
            nc.gpsimd.dma_start(out=woT_sb, in_=woT_d.ap())

        pair_unit(0, 1, [VU(2, 0), VU(2, 1), VU(2, 2), VU(2, 3)],
                  [VU(3, 0), VU(3, 1), VU(3, 2), VU(3, 3)])
        pair_unit(1, 1, [QK(2, 2), QK(2, 3), QK(6, 2)], [QK(6, 3)])
        pair_unit(2, 1, [QK(3, 2), QK(3, 3), QK(7, 2)],
                  [QK(7, 3), load_wo, OP(0, 0), OP(0, 1), OP(1, 0), OP(1, 1)])
        pair_unit(3, 1, [OP(2, 0), OP(2, 1), OP(3, 0), OP(3, 1),
                         OP(4, 0), OP(4, 1), OP(5, 0), OP(5, 1)],
                  [OP(6, 0), OP(6, 1), OP(7, 0), OP(7, 1),
                   OP(8, 0), OP(8, 1), OP(9, 0), OP(9, 1),
                   OP(10, 0), OP(10, 1), OP(11, 0), OP(11, 1)],
                  inline_norm=True)

        # drain deferred normalizes, then the tail of the output projection
        while dq:
            dq.pop(0)()
        for lt in range(12, LT):
            for ec in range(2):
                op_unit(lt, ec, ring=nc.sync)

    nc.compile()
    return nc


def _shuf(a):
    """[n*128, C] -> [128, n, C] (partition-major, contiguous per partition)."""
    R, C = a.shape
    return np.ascontiguousarray(a.reshape(R // 128, 128, C).transpose(1, 0, 2))


def make_in_maps(x, w_qkv, wo):
    """Host-side sharding: 8 cores = (batch b=c//2, head-group g=c%2)."""
    import ml_dtypes
    bf = ml_dtypes.bfloat16
    x = np.asarray(x, dtype=np.float32)
    w_qkv = np.asarray(w_qkv, dtype=np.float32)
    wo = np.asarray(wo, dtype=np.float32)
    diag = np.triu(np.ones((128, 128), np.float32)).astype(bf)
    in_maps = []
    for c in range(8):
        b, g = c // 2, c % 2
        js = slice(g * JQ, (g + 1) * JQ)
        wq = w_qkv[0:E][js]
        wk = w_qkv[E:2 * E][js]
        wv = w_qkv[2 * E:3 * E][js]
        xT = x[b].T.astype(bf)                                   # [E, L]
        wqkvT = np.concatenate([wq, wk, wv], 0).T.astype(bf)     # [E, 3JQ]
        m = {
            "wqkT": _shuf(wqkvT[:, 0:2 * JQ]),
            "wvT": _shuf(wqkvT[:, 2 * JQ:3 * JQ]),
            "woT": _shuf(wo[:, js].T.astype(bf)),
            "diag": diag,
        }
        for cc in range(4):
            m[f"xT{cc}"] = _shuf(xT[:, cc * 512:(cc + 1) * 512])
        in_maps.append(m)
    return in_maps


def _get_nc():
    if "nc" not in _CACHE:
        _CACHE["nc"] = build_nc()
    return _CACHE["nc"]


def kernel(x, mask, w_qkv, wo, _trace=False, _trace_kwargs=None):
    from concourse.bass_utils import run_bass_kernel_spmd

    nc = _get_nc()
    in_maps = make_in_maps(x, w_qkv, wo)
    res = run_bass_kernel_spmd(
        nc, in_maps, core_ids=list(range(8)),
        trace=_trace, **(_trace_kwargs or {}),
    )
    _CACHE["last_results"] = res
    y = np.stack([res.results[2 * b]["y"] + res.results[2 * b + 1]["y"] for b in range(4)])
    return y.astype(np.float32)

